# revision 7
# baseline (speedup 1.0000x reference)
"""CoPEGate Trainium2 kernel.

Computes out[b,h,t,s] = sigmoid((Q K^T)[b,h,t,s] / sqrt(D)) * (P P^T)[t,s] / sqrt(D)
for B=2, H=12, T=2048, D=64 (fp32), distributed over 8 NeuronCores.

Sharding: the 24 (b,h) pairs are split 3-per-core (head-parallel); the
positional matrix P is replicated and its T x T bias is computed on every
core (it is reused across that core's 3 heads). No cross-device
communication is needed.

Per-core dataflow (all shapes per core):
  inputs   qT, kT: [3, 64, 2048] bf16 (host pre-transposed so the matmul
           contraction dim D=64 lands on SBUF partitions, and host-cast to
           bf16 for full-rate matmuls), pT: [64, 2048] fp32 (f32r matmul,
           ~1.5e-4 rel err, so the pos bias factor stays near-fp32).
  loop over 16 row-tiles of 128:
    pos stripe   = PE matmul pT[:, tile].T @ pT          -> PSUM [128, 2048]
                   ScalarE Copy * 1/sqrt(D)              -> SBUF
    per head h:  scores = PE matmul qT[h][:, tile].T @ kT[h] -> PSUM
                 gate   = ScalarE Sigmoid(scores / sqrt(D))  -> SBUF
                 out    = VectorE gate * pos stripe          -> SBUF slice
    one 3 MiB DMA per row-tile writes all 3 heads' stripes -> HBM
The kernel is output-DMA bound (~50 MiB written per core, ~358 GB/s/core).

PE utilization trick: the K=64 contraction only uses half the 128-row PE
array, so operands are laid out in alternating partition halves --
pos & head0 at partitions 0-63, heads 1 & 2 at partitions 64-127 -- and
stripes are issued in the order pos, h1, h0, h2. Adjacent matmuls then
target disjoint PE row groups (bass auto-derives tile_position from the
operand base partition) and can execute concurrently in the array.
"""

import math
import os
import sys

import numpy as np

sys.path.insert(0, "/opt/trn_rl_repo")

B, H, T, D = 2, 12, 2048, 64
N_CORES = 8
HPC = (B * H) // N_CORES  # heads per core
PT = 128  # output row-tile height (SBUF/PSUM partitions)
NT = T // PT  # row tiles
NCHUNK = 512  # matmul moving-operand free dim (one PSUM bank of fp32)
NCH = T // NCHUNK
INV_SQRT_D = 1.0 / math.sqrt(D)

_NC_CACHE = {}


def _build_nc():
    import concourse.bass as bass
    from concourse import bacc, mybir, tile

    f32 = mybir.dt.float32
    f32r = mybir.dt.float32r
    bf16 = mybir.dt.bfloat16
    Sigmoid = mybir.ActivationFunctionType.Sigmoid
    Copy = mybir.ActivationFunctionType.Copy

    nc = bacc.Bacc("TRN2", target_bir_lowering=False)

    qT = nc.dram_tensor("qT", [HPC, D, T], bf16, kind="ExternalInput")
    kT = nc.dram_tensor("kT", [HPC, D, T], bf16, kind="ExternalInput")
    pT = nc.dram_tensor("pT", [D, T], f32r, kind="ExternalInput")
    out = nc.dram_tensor("out", [HPC, T, T], f32, kind="ExternalOutput")

    with tile.TileContext(nc) as tc:
        with tc.tile_pool(name="ins", bufs=1) as ins_pool, \
             tc.tile_pool(name="pos", bufs=2) as pos_pool, \
             tc.tile_pool(name="gate", bufs=4) as gate_pool, \
             tc.tile_pool(name="outs", bufs=6) as outs_pool, \
             tc.tile_pool(name="ps", bufs=2, space="PSUM") as ps_pool:

            # Load order matters for pipeline ramp-up: the first stripes
            # need pT (pos bias) and head 0's q/k, so load those first.
            # Heads 0+1 are one contiguous [128, 2048] DMA (full port BW);
            # head 2 goes to partitions 64-127 of its own tile so that its
            # matmuls use the upper PE row group.
            p_sb = ins_pool.tile([D, T], f32r, tag="p")
            nc.sync.dma_start(out=p_sb, in_=pT[:])
            k01 = ins_pool.tile([2 * D, T], bf16, tag="k01")
            nc.sync.dma_start(out=k01, in_=kT[0:2].rearrange("h d t -> (h d) t"))
            q01 = ins_pool.tile([2 * D, T], bf16, tag="q01")
            nc.sync.dma_start(out=q01, in_=qT[0:2].rearrange("h d t -> (h d) t"))
            k2 = ins_pool.tile([2 * D, T], bf16, tag="k2")
            nc.sync.dma_start(out=k2[D : 2 * D, :], in_=kT[2])
            q2 = ins_pool.tile([2 * D, T], bf16, tag="q2")
            nc.sync.dma_start(out=q2[D : 2 * D, :], in_=qT[2])

            # Per-head operand slices; heads 1, 2 live at base partition 64
            # (upper PE row group), head 0 and pos at base partition 0.
            q_sb = [q01[0:D, :], q01[D : 2 * D, :], q2[D : 2 * D, :]]
            k_sb = [k01[0:D, :], k01[D : 2 * D, :], k2[D : 2 * D, :]]
            # Issue order alternates PE row groups: pos(lo), h1(hi),
            # h0(lo), h2(hi) -- adjacent matmuls can overlap in the array.
            head_order = [1, 0, 2]

            for it in range(NT):
                tsl = bass.ts(it, PT)
                # The first and last row-tiles post-process in 512-column
                # chunks: ACT/DVE/DMA start as soon as the first PSUM bank
                # is written, shrinking pipeline ramp-up and drain. Steady
                # state uses full 2048-wide ops (lower per-op overhead).
                npost = NCH if it in (0, NT - 1) else 1
                cw = T // npost  # post-processing chunk width

                pp = ps_pool.tile([PT, T], f32, tag="ps")
                for j in range(NCH):
                    nc.tensor.matmul(
                        pp[:, bass.ts(j, NCHUNK)],
                        p_sb[:, tsl],
                        p_sb[:, bass.ts(j, NCHUNK)],
                        start=True,
                        stop=True,
                    )
                pos_sb = pos_pool.tile([PT, T], f32, tag="pos")
                for c in range(npost):
                    nc.scalar.activation(
                        pos_sb[:, bass.ts(c, cw)],
                        pp[:, bass.ts(c, cw)],
                        Copy,
                        scale=INV_SQRT_D,
                    )

                for h in head_order:
                    sp = ps_pool.tile([PT, T], f32, tag="ps")
                    for j in range(NCH):
                        nc.tensor.matmul(
                            sp[:, bass.ts(j, NCHUNK)],
                            q_sb[h][:, tsl],
                            k_sb[h][:, bass.ts(j, NCHUNK)],
                            start=True,
                            stop=True,
                        )
                    gate = gate_pool.tile([PT, T], f32, tag="gate")
                    o = outs_pool.tile([PT, T], f32, tag="o")
                    for c in range(npost):
                        csl = bass.ts(c, cw)
                        nc.scalar.activation(
                            gate[:, csl], sp[:, csl], Sigmoid, scale=INV_SQRT_D
                        )
                        nc.vector.tensor_mul(o[:, csl], gate[:, csl], pos_sb[:, csl])
                        nc.sync.dma_start(
                            out=out[h, tsl, csl], in_=o[:, csl]
                        )

    nc.finalize()
    return nc


def _get_nc():
    if "nc" not in _NC_CACHE:
        _NC_CACHE["nc"] = _build_nc()
    return _NC_CACHE["nc"]


def kernel(query, key, pos_embed_weight):
    import ml_dtypes

    query = np.asarray(query, dtype=np.float32)
    key = np.asarray(key, dtype=np.float32)
    pos_embed_weight = np.asarray(pos_embed_weight, dtype=np.float32)

    q = query.reshape(B * H, T, D)
    k = key.reshape(B * H, T, D)
    p_t = np.ascontiguousarray(pos_embed_weight[:T].T)  # [D, T]

    bf = ml_dtypes.bfloat16
    in_maps = []
    for c in range(N_CORES):
        hs = slice(c * HPC, (c + 1) * HPC)
        in_maps.append(
            {
                "qT": np.ascontiguousarray(
                    q[hs].transpose(0, 2, 1).astype(bf)
                ),
                "kT": np.ascontiguousarray(
                    k[hs].transpose(0, 2, 1).astype(bf)
                ),
                "pT": p_t,
            }
        )

    from concourse.bass_utils import run_bass_kernel_spmd

    nc = _get_nc()
    res = run_bass_kernel_spmd(
        nc,
        in_maps,
        core_ids=list(range(N_CORES)),
        trace=bool(os.environ.get("KERNEL_TRACE")),
    )
    kernel.last_results = res

    full = np.empty((B * H, T, T), dtype=np.float32)
    for c in range(N_CORES):
        full[c * HPC : (c + 1) * HPC] = res.results[c]["out"]
    return full.reshape(B, H, T, T)


kernel.last_results = None
